# revision 5
# baseline (speedup 1.0000x reference)
"""Trainium2 Bass kernel for the batched bilinear layer:

    out[b,r] = sigmoid( (E @ W_r) @ E^T ),  E = inputs[b,r] : [N=1024, F=256],
    W_r = Bs[r] : [256, 256]

Sharding: batch dim B=8 across the 8 NeuronCores (data parallel). Each core
receives x = inputs[b] : [R=8, 1024, 256] plus the full (replicated) Bs, and
produces y = out[b] : [R=8, 1024, 1024].

Per (b, r) on-chip dataflow (all contractions are over E's feature axis, so E
is needed feature-on-partitions both times -> one transpose of E per (b,r)):
  1. load E naturally ([n,f] tiles), PE-transpose 128x128 blocks -> ET [f|g, n]
  2. eT[g,n]  = sum_f W[f,g] * ET[f,n]     (lhsT = W natural, rhs = ET)
  3. s[n,m]   = sum_g eT[g,n] * ET[g,m]    (lhsT = eT,        rhs = ET)
  4. sigmoid via ScalarE (ACT) straight out of PSUM, DMA to DRAM.

Matmuls run as float32r (single-pass fp32, 1 cycle/row at free-dim>=256)
instead of float32 (2-pass, 4 cycles/row).
"""

from contextlib import ExitStack

import numpy as np

import concourse.bass as bass
import concourse.mybir as mybir
import concourse.tile as tile
from concourse import bacc
from concourse.bass_utils import run_bass_kernel_spmd
from concourse.masks import make_identity

B, R, N, F = 8, 8, 1024, 256
NCORES = 8
P = 128  # partitions
F32 = mybir.dt.float32
F32R = mybir.dt.float32r

NB = N // P   # 8  n-blocks of 128
FB = F // P   # 2  f/g-blocks of 128
NC_ = N // 512  # 2  512-wide chunks of the moving dim

_cache = {}


def build_bass(out_dtype=F32):
    nc = bacc.Bacc(
        "TRN2", target_bir_lowering=False, debug=False, num_devices=NCORES
    )
    x = nc.declare_dram_parameter("x", [R, N, F], F32, isOutput=False)
    w = nc.declare_dram_parameter("w", [R, F, F], F32, isOutput=False)
    y = nc.declare_dram_parameter("y", [R, N, N], out_dtype, isOutput=True)

    with ExitStack() as ctx:
        tc = ctx.enter_context(tile.TileContext(nc))
        const_pool = ctx.enter_context(tc.tile_pool(name="const", bufs=1))
        ident = const_pool.tile([P, P], F32)
        make_identity(nc, ident)

        en_pool = ctx.enter_context(tc.tile_pool(name="en", bufs=4))
        et_pool = ctx.enter_context(tc.tile_pool(name="et", bufs=2 * FB))
        w_pool = ctx.enter_context(tc.tile_pool(name="wp", bufs=2 * FB))
        e2_pool = ctx.enter_context(tc.tile_pool(name="e2", bufs=2 * FB))
        sig_pool = ctx.enter_context(tc.tile_pool(name="sig", bufs=6))
        pt_pool = ctx.enter_context(tc.tile_pool(name="pt", bufs=2, space="PSUM"))
        pe_pool = ctx.enter_context(tc.tile_pool(name="pe", bufs=2, space="PSUM"))
        ps_pool = ctx.enter_context(tc.tile_pool(name="ps", bufs=2, space="PSUM"))

        for r in range(R):
            # ---- load weights W_r as [f, g] (natural layout), round to f32r ----
            wt = []
            for k in range(FB):
                ws = en_pool.tile([P, F], F32, name=f"ws_{r}_{k}", tag="en")
                nc.sync.dma_start(out=ws, in_=w[r, P * k : P * (k + 1), :])
                wk = w_pool.tile([P, F], F32R, name=f"w_{r}_{k}", tag="w")
                nc.vector.tensor_copy(wk, ws)
                wt.append(wk)

            # ---- load E naturally, transpose to ET[j] : [128 f, 1024 n] ----
            ET = [
                et_pool.tile([P, N], F32R, name=f"et_{r}_{j}", tag="et")
                for j in range(FB)
            ]
            for i in range(NB):
                en = en_pool.tile([P, F], F32, name=f"en_{r}_{i}", tag="en")
                nc.sync.dma_start(out=en, in_=x[r, P * i : P * (i + 1), :])
                for j in range(FB):
                    pt = pt_pool.tile([P, P], F32, name=f"pt_{r}_{i}_{j}", tag="pt")
                    nc.tensor.transpose(pt, en[:, P * j : P * (j + 1)], ident)
                    nc.vector.tensor_copy(ET[j][:, P * i : P * (i + 1)], pt)

            # ---- matmul 1: eT[j] = (E @ W).T chunks  [128 g, 1024 n] ----
            E2 = []
            for j in range(FB):
                e2 = e2_pool.tile([P, N], F32R, name=f"e2_{r}_{j}", tag="e2")
                for c in range(NC_):
                    pe = pe_pool.tile([P, 512], F32, name=f"pe_{r}_{j}_{c}", tag="pe")
                    for k in range(FB):
                        nc.tensor.matmul(
                            pe,
                            lhsT=wt[k][:, P * j : P * (j + 1)],
                            rhs=ET[k][:, 512 * c : 512 * (c + 1)],
                            start=(k == 0),
                            stop=(k == FB - 1),
                        )
                    nc.vector.tensor_copy(e2[:, 512 * c : 512 * (c + 1)], pe)
                E2.append(e2)

            # ---- matmul 2 + sigmoid + store, one 128-row output block at a time ----
            for i in range(NB):
                ps = ps_pool.tile([P, N], F32, name=f"ps_{r}_{i}", tag="ps")
                for c in range(NC_):
                    for j in range(FB):
                        nc.tensor.matmul(
                            ps[:, 512 * c : 512 * (c + 1)],
                            lhsT=E2[j][:, P * i : P * (i + 1)],
                            rhs=ET[j][:, 512 * c : 512 * (c + 1)],
                            start=(j == 0),
                            stop=(j == FB - 1),
                        )
                sg = sig_pool.tile([P, N], out_dtype, name=f"sg_{r}_{i}", tag="sg")
                nc.scalar.activation(sg, ps, mybir.ActivationFunctionType.Sigmoid)
                nc.scalar.dma_start(out=y[r, P * i : P * (i + 1), :], in_=sg)

    nc.compile()
    return nc


def kernel(inputs: np.ndarray, Bs: np.ndarray) -> np.ndarray:
    inputs = np.ascontiguousarray(inputs, dtype=np.float32)
    Bs = np.ascontiguousarray(Bs, dtype=np.float32)
    if "nc" not in _cache:
        _cache["nc"] = build_bass()
    nc = _cache["nc"]
    in_maps = [{"x": inputs[c], "w": Bs} for c in range(NCORES)]
    res = run_bass_kernel_spmd(nc, in_maps, list(range(NCORES)))
    out = np.stack([res.results[c]["y"] for c in range(NCORES)], axis=0)
    return out.astype(np.float32, copy=False)


# revision 7
# speedup vs baseline: 1.0943x; 1.0943x over previous
"""Trainium2 Bass kernel for the batched bilinear layer:

    out[b,r] = sigmoid( (E @ W_r) @ E^T ),  E = inputs[b,r] : [N=1024, F=256],
    W_r = Bs[r] : [256, 256]

Sharding: batch dim B=8 across the 8 NeuronCores (data parallel). Each core
receives x = inputs[b] : [R=8, 1024, 256] plus the full (replicated) Bs, and
produces y = out[b] : [R=8, 1024, 1024].

Per (b, r) on-chip dataflow (all contractions are over E's feature axis, so E
is needed feature-on-partitions both times -> one transpose of E per (b,r)):
  1. load E naturally ([n,f] tiles), PE-transpose 128x128 blocks -> ET [f|g, n]
  2. eT[g,n]  = sum_f W[f,g] * ET[f,n]     (lhsT = W natural, rhs = ET)
  3. s[n,m]   = sum_g eT[g,n] * ET[g,m]    (lhsT = eT,        rhs = ET)
  4. sigmoid via ScalarE (ACT) straight out of PSUM, DMA to DRAM.

Matmuls run as float32r (single-pass fp32, 1 cycle/row at free-dim>=256)
instead of float32 (2-pass, 4 cycles/row). Transposes are batched 4-to-a-PSUM-
bank so one DVE cast moves a [128,512] chunk into the f32r ET tiles.
"""

from contextlib import ExitStack

import numpy as np

import concourse.bass as bass
import concourse.mybir as mybir
import concourse.tile as tile
from concourse import bacc
from concourse.bass_utils import run_bass_kernel_spmd
from concourse.masks import make_identity

B, R, N, F = 8, 8, 1024, 256
NCORES = 8
P = 128  # partitions
F32 = mybir.dt.float32
F32R = mybir.dt.float32r
BF16 = mybir.dt.bfloat16

NB = N // P     # 8  n-blocks of 128
FB = F // P     # 2  f/g-blocks of 128
CH = N // 512   # 2  512-wide chunks of the n/m dim

_cache = {}


def build_bass(out_bf16=False):
    out_dtype = BF16 if out_bf16 else F32
    nc = bacc.Bacc(
        "TRN2", target_bir_lowering=False, debug=False, num_devices=NCORES
    )
    x = nc.declare_dram_parameter("x", [R, N, F], F32, isOutput=False)
    w = nc.declare_dram_parameter("w", [R, F, F], F32, isOutput=False)
    y = nc.declare_dram_parameter("y", [R, N, N], out_dtype, isOutput=True)

    with ExitStack() as ctx:
        tc = ctx.enter_context(tile.TileContext(nc))
        const_pool = ctx.enter_context(tc.tile_pool(name="const", bufs=1))
        ident = const_pool.tile([P, P], F32)
        make_identity(nc, ident)

        # ---- preload all weights W_r as [f, g] f32r, resident whole kernel ----
        wst_pool = ctx.enter_context(tc.tile_pool(name="wst", bufs=2))
        w_pool = ctx.enter_context(tc.tile_pool(name="wp", bufs=1))
        WT = []
        for r in range(R):
            wr = []
            for k in range(FB):
                ws = wst_pool.tile([P, F], F32, name=f"ws_{r}_{k}", tag="ws")
                nc.sync.dma_start(out=ws, in_=w[r, P * k : P * (k + 1), :])
                wk = w_pool.tile([P, F], F32R, name=f"w_{r}_{k}", tag=f"w_{r}_{k}")
                nc.vector.tensor_copy(wk, ws)
                wr.append(wk)
            WT.append(wr)

        en_pool = ctx.enter_context(tc.tile_pool(name="en", bufs=3))
        et_pool = ctx.enter_context(tc.tile_pool(name="et", bufs=2 * FB))
        e2_pool = ctx.enter_context(tc.tile_pool(name="e2", bufs=2 * FB))
        sig_pool = ctx.enter_context(tc.tile_pool(name="sig", bufs=6))
        pt_pool = ctx.enter_context(tc.tile_pool(name="pt", bufs=2, space="PSUM"))
        pe_pool = ctx.enter_context(tc.tile_pool(name="pe", bufs=2, space="PSUM"))
        ps_pool = ctx.enter_context(tc.tile_pool(name="ps", bufs=2, space="PSUM"))

        for r in range(R):
            # ---- load E in two [128, 1024] tiles (4 n-blocks each),
            #      transpose 4 blocks per PSUM bank, cast to ET chunks ----
            ET = [
                et_pool.tile([P, N], F32R, name=f"et_{r}_{j}", tag="et")
                for j in range(FB)
            ]
            for c in range(CH):
                en = en_pool.tile([P, 4 * F], F32, name=f"en_{r}_{c}", tag="en")
                nc.sync.dma_start(
                    out=en.rearrange("p (u f) -> p u f", u=4),
                    in_=x[r, 512 * c : 512 * (c + 1), :].rearrange(
                        "(u p) f -> p u f", p=P
                    ),
                )
                for j in range(FB):
                    pt = pt_pool.tile([P, 512], F32, name=f"pt_{r}_{c}_{j}", tag="pt")
                    for u in range(4):
                        nc.tensor.transpose(
                            pt[:, P * u : P * (u + 1)],
                            en[:, F * u + P * j : F * u + P * (j + 1)],
                            ident,
                        )
                    nc.vector.tensor_copy(ET[j][:, 512 * c : 512 * (c + 1)], pt)

            # ---- matmul 1: eT[j] = (E @ W).T chunks  [128 g, 1024 n] ----
            E2 = []
            for j in range(FB):
                e2 = e2_pool.tile([P, N], F32R, name=f"e2_{r}_{j}", tag="e2")
                for c in range(CH):
                    pe = pe_pool.tile([P, 512], F32, name=f"pe_{r}_{j}_{c}", tag="pe")
                    for k in range(FB):
                        nc.tensor.matmul(
                            pe,
                            lhsT=WT[r][k][:, P * j : P * (j + 1)],
                            rhs=ET[k][:, 512 * c : 512 * (c + 1)],
                            start=(k == 0),
                            stop=(k == FB - 1),
                        )
                    nc.vector.tensor_copy(e2[:, 512 * c : 512 * (c + 1)], pe)
                E2.append(e2)

            # ---- matmul 2 + sigmoid + store, one 128-row output block at a time ----
            for i in range(NB):
                ps = ps_pool.tile([P, N], F32, name=f"ps_{r}_{i}", tag="ps")
                for c in range(CH):
                    for j in range(FB):
                        nc.tensor.matmul(
                            ps[:, 512 * c : 512 * (c + 1)],
                            lhsT=E2[j][:, P * i : P * (i + 1)],
                            rhs=ET[j][:, 512 * c : 512 * (c + 1)],
                            start=(j == 0),
                            stop=(j == FB - 1),
                        )
                sg = sig_pool.tile([P, N], out_dtype, name=f"sg_{r}_{i}", tag="sg")
                nc.scalar.activation(sg, ps, mybir.ActivationFunctionType.Sigmoid)
                nc.scalar.dma_start(out=y[r, P * i : P * (i + 1), :], in_=sg)

    nc.compile()
    return nc


OUT_BF16 = True


def kernel(inputs: np.ndarray, Bs: np.ndarray) -> np.ndarray:
    inputs = np.ascontiguousarray(inputs, dtype=np.float32)
    Bs = np.ascontiguousarray(Bs, dtype=np.float32)
    key = ("nc", OUT_BF16)
    if key not in _cache:
        _cache[key] = build_bass(out_bf16=OUT_BF16)
    nc = _cache[key]
    in_maps = [{"x": inputs[c], "w": Bs} for c in range(NCORES)]
    res = run_bass_kernel_spmd(nc, in_maps, list(range(NCORES)))
    out = np.stack([res.results[c]["y"] for c in range(NCORES)], axis=0)
    return out.astype(np.float32, copy=False)


# revision 8
# speedup vs baseline: 1.5694x; 1.4342x over previous
"""Trainium2 Bass kernel for the batched bilinear layer:

    out[b,r] = sigmoid( (E @ W_r) @ E^T ),  E = inputs[b,r] : [N=1024, F=256],
    W_r = Bs[r] : [256, 256]

Sharding: batch dim B=8 across the 8 NeuronCores (data parallel). Each core
receives x = inputs[b] : [R=8, 1024, 256] plus the full (replicated) Bs, and
produces y = out[b] : [R=8, 1024, 1024].

Pipeline (all-fp16 operands, fp32 PSUM accumulation):
  1. E loads as fp16 via gpsimd casting DMA, naturally ([n, f] layout).
  2. PE-transpose (fp16, 1 cyc/row) 128x128 blocks into one PSUM bank per
     feature-half; one DVE copy moves each [128, 1024] ET row to SBUF.
  3. eT[g,n] = sum_f W[f,g] ET[f,n]   (lhsT = W fp16, rhs = ET fp16)
  4. s[n,m]  = sum_g eT[g,n] ET[g,m]  (lhsT = eT fp16, rhs = ET fp16)
  5. sigmoid on ScalarE from PSUM -> fp16 SBUF -> DMA (sync ring) to DRAM.

fp16 (10-bit mantissa) matches fp32r precision while halving LDWEIGHTS cost
(the Tensor sequencer was co-critical with the PE array in the f32r version)
and keeping the PE at 1 cycle/row everywhere.
"""

from contextlib import ExitStack

import numpy as np

import concourse.bass as bass
import concourse.mybir as mybir
import concourse.tile as tile
from concourse import bacc
from concourse.bass_utils import run_bass_kernel_spmd
from concourse.masks import make_identity

B, R, N, F = 8, 8, 1024, 256
NCORES = 8
P = 128  # partitions
F32 = mybir.dt.float32
F16 = mybir.dt.float16

NB = N // P     # 8  n-blocks of 128
FB = F // P     # 2  f/g-blocks of 128
CH = N // 512   # 2  512-wide chunks of the n/m dim

_cache = {}


def build_bass():
    nc = bacc.Bacc(
        "TRN2", target_bir_lowering=False, debug=False, num_devices=NCORES
    )
    x = nc.declare_dram_parameter("x", [R, N, F], F32, isOutput=False)
    w = nc.declare_dram_parameter("w", [R, F, F], F32, isOutput=False)
    y = nc.declare_dram_parameter("y", [R, N, N], F16, isOutput=True)

    with ExitStack() as ctx:
        tc = ctx.enter_context(tile.TileContext(nc))
        const_pool = ctx.enter_context(tc.tile_pool(name="const", bufs=1))
        ident = const_pool.tile([P, P], F16)
        make_identity(nc, ident)

        # ---- preload all weights W_r as [f, g] fp16, resident whole kernel ----
        wst_pool = ctx.enter_context(tc.tile_pool(name="wst", bufs=2))
        w_pool = ctx.enter_context(tc.tile_pool(name="wp", bufs=1))
        WT = []
        for r in range(R):
            wr = []
            for k in range(FB):
                ws = wst_pool.tile([P, F], F32, name=f"ws_{r}_{k}", tag="ws")
                nc.sync.dma_start(out=ws, in_=w[r, P * k : P * (k + 1), :])
                wk = w_pool.tile([P, F], F16, name=f"w_{r}_{k}", tag=f"w_{r}_{k}")
                nc.vector.tensor_copy(wk, ws)
                wr.append(wk)
            WT.append(wr)

        en_pool = ctx.enter_context(tc.tile_pool(name="en", bufs=4))
        et_pool = ctx.enter_context(tc.tile_pool(name="et", bufs=2 * FB))
        e2_pool = ctx.enter_context(tc.tile_pool(name="e2", bufs=2 * FB))
        sig_pool = ctx.enter_context(tc.tile_pool(name="sig", bufs=6))
        pt_pool = ctx.enter_context(tc.tile_pool(name="pt", bufs=2, space="PSUM"))
        pe_pool = ctx.enter_context(tc.tile_pool(name="pe", bufs=2, space="PSUM"))
        ps_pool = ctx.enter_context(tc.tile_pool(name="ps", bufs=2, space="PSUM"))

        for r in range(R):
            # ---- load E as fp16 (gpsimd casting DMA), [128, 4, 256] per chunk ----
            ens = []
            for c in range(CH):
                en = en_pool.tile([P, 4 * F], F16, name=f"en_{r}_{c}", tag="en")
                nc.gpsimd.dma_start(
                    out=en.rearrange("p (u f) -> p u f", u=4),
                    in_=x[r, 512 * c : 512 * (c + 1), :].rearrange(
                        "(u p) f -> p u f", p=P
                    ),
                )
                ens.append(en)

            # ---- transpose: 8 blocks into one PSUM bank per j, 1 DVE copy each ----
            ET = [
                et_pool.tile([P, N], F16, name=f"et_{r}_{j}", tag="et")
                for j in range(FB)
            ]
            for j in range(FB):
                pt = pt_pool.tile([P, N], F16, name=f"pt_{r}_{j}", tag="pt")
                for c in range(CH):
                    for u in range(4):
                        nc.tensor.transpose(
                            pt[:, 512 * c + P * u : 512 * c + P * (u + 1)],
                            ens[c][:, F * u + P * j : F * u + P * (j + 1)],
                            ident,
                        )
                nc.vector.tensor_copy(ET[j], pt)

            # ---- matmul 1: eT[j] = (E @ W).T chunks  [128 g, 1024 n] ----
            E2 = []
            for j in range(FB):
                e2 = e2_pool.tile([P, N], F16, name=f"e2_{r}_{j}", tag="e2")
                for c in range(CH):
                    pe = pe_pool.tile([P, 512], F32, name=f"pe_{r}_{j}_{c}", tag="pe")
                    for k in range(FB):
                        nc.tensor.matmul(
                            pe,
                            lhsT=WT[r][k][:, P * j : P * (j + 1)],
                            rhs=ET[k][:, 512 * c : 512 * (c + 1)],
                            start=(k == 0),
                            stop=(k == FB - 1),
                        )
                    nc.vector.tensor_copy(e2[:, 512 * c : 512 * (c + 1)], pe)
                E2.append(e2)

            # ---- matmul 2 + sigmoid + store, one 128-row output block at a time ----
            for i in range(NB):
                ps = ps_pool.tile([P, N], F32, name=f"ps_{r}_{i}", tag="ps")
                for c in range(CH):
                    for j in range(FB):
                        nc.tensor.matmul(
                            ps[:, 512 * c : 512 * (c + 1)],
                            lhsT=E2[j][:, P * i : P * (i + 1)],
                            rhs=ET[j][:, 512 * c : 512 * (c + 1)],
                            start=(j == 0),
                            stop=(j == FB - 1),
                        )
                sg = sig_pool.tile([P, N], F16, name=f"sg_{r}_{i}", tag="sg")
                nc.scalar.activation(sg, ps, mybir.ActivationFunctionType.Sigmoid)
                nc.sync.dma_start(out=y[r, P * i : P * (i + 1), :], in_=sg)

    nc.compile()
    return nc


def kernel(inputs: np.ndarray, Bs: np.ndarray) -> np.ndarray:
    inputs = np.ascontiguousarray(inputs, dtype=np.float32)
    Bs = np.ascontiguousarray(Bs, dtype=np.float32)
    if "nc" not in _cache:
        _cache["nc"] = build_bass()
    nc = _cache["nc"]
    in_maps = [{"x": inputs[c], "w": Bs} for c in range(NCORES)]
    res = run_bass_kernel_spmd(nc, in_maps, list(range(NCORES)))
    out = np.stack(
        [np.asarray(res.results[c]["y"], dtype=np.float32) for c in range(NCORES)],
        axis=0,
    )
    return out


# revision 9
# speedup vs baseline: 1.6917x; 1.0780x over previous
"""Trainium2 Bass kernel for the batched bilinear layer:

    out[b,r] = sigmoid( (E @ W_r) @ E^T ),  E = inputs[b,r] : [N=1024, F=256],
    W_r = Bs[r] : [256, 256]

Sharding: batch dim B=8 across the 8 NeuronCores (data parallel). Each core
receives x = inputs[b] : [R=8, 1024, 256] plus the full (replicated) Bs, and
produces y = out[b] : [R=8, 1024, 1024].

Pipeline (all-fp16 operands, fp32 PSUM accumulation):
  1. E loads as fp16 via gpsimd casting DMA, naturally ([n, f] layout).
  2. PE-transpose (fp16, 1 cyc/row) 128x128 blocks into one PSUM bank per
     feature-half; one DVE copy moves each [128, 1024] ET row to SBUF.
  3. eT[g,n] = sum_f W[f,g] ET[f,n]   (lhsT = W fp16, rhs = ET fp16)
  4. s[n,m]  = sum_g eT[g,n] ET[g,m]  (lhsT = eT fp16, rhs = ET fp16)
  5. sigmoid on ScalarE from PSUM -> fp16 SBUF -> DMA (sync ring) to DRAM.

fp16 (10-bit mantissa) matches fp32r precision while halving LDWEIGHTS cost
(the Tensor sequencer was co-critical with the PE array in the f32r version)
and keeping the PE at 1 cycle/row everywhere.
"""

from contextlib import ExitStack

import numpy as np

import concourse.bass as bass
import concourse.mybir as mybir
import concourse.tile as tile
from concourse import bacc
from concourse.bass_utils import run_bass_kernel_spmd
from concourse.masks import make_identity

B, R, N, F = 8, 8, 1024, 256
NCORES = 8
P = 128  # partitions
F32 = mybir.dt.float32
F16 = mybir.dt.float16

NB = N // P     # 8  n-blocks of 128
FB = F // P     # 2  f/g-blocks of 128
CH = N // 512   # 2  512-wide chunks of the n/m dim

_cache = {}


def build_bass():
    nc = bacc.Bacc(
        "TRN2", target_bir_lowering=False, debug=False, num_devices=NCORES
    )
    x = nc.declare_dram_parameter("x", [R, N, F], F32, isOutput=False)
    w = nc.declare_dram_parameter("w", [R, F, F], F32, isOutput=False)
    y = nc.declare_dram_parameter("y", [R, N, N], F16, isOutput=True)

    with ExitStack() as ctx:
        tc = ctx.enter_context(tile.TileContext(nc))
        const_pool = ctx.enter_context(tc.tile_pool(name="const", bufs=1))
        ident = const_pool.tile([P, P], F16)
        make_identity(nc, ident)

        # ---- weights W_r as [f, g] fp16 via gpsimd casting DMA (no staging),
        #      resident whole kernel; W_0 up front, W_{r+1} prefetched in-loop ----
        w_pool = ctx.enter_context(tc.tile_pool(name="wp", bufs=1))

        def load_w(r):
            wr = []
            for k in range(FB):
                wk = w_pool.tile([P, F], F16, name=f"w_{r}_{k}", tag=f"w_{r}_{k}")
                nc.gpsimd.dma_start(out=wk, in_=w[r, P * k : P * (k + 1), :])
                wr.append(wk)
            return wr

        en_pool = ctx.enter_context(tc.tile_pool(name="en", bufs=4))
        et_pool = ctx.enter_context(tc.tile_pool(name="et", bufs=5))
        e2_pool = ctx.enter_context(tc.tile_pool(name="e2", bufs=5))
        sig_pool = ctx.enter_context(tc.tile_pool(name="sig", bufs=6))
        pt_pool = ctx.enter_context(tc.tile_pool(name="pt", bufs=2, space="PSUM"))
        pe_pool = ctx.enter_context(tc.tile_pool(name="pe", bufs=2, space="PSUM"))
        ps_pool = ctx.enter_context(tc.tile_pool(name="ps", bufs=2, space="PSUM"))

        WT = [None] * R
        for r in range(R):
            # ---- load E as fp16 (gpsimd casting DMA), [128, 4, 256] per chunk ----
            ens = []
            for c in range(CH):
                en = en_pool.tile([P, 4 * F], F16, name=f"en_{r}_{c}", tag="en")
                nc.gpsimd.dma_start(
                    out=en.rearrange("p (u f) -> p u f", u=4),
                    in_=x[r, 512 * c : 512 * (c + 1), :].rearrange(
                        "(u p) f -> p u f", p=P
                    ),
                )
                ens.append(en)
            if r == 0:
                WT[0] = load_w(0)
            if r + 1 < R:
                WT[r + 1] = load_w(r + 1)

            # ---- transpose: 8 blocks into one PSUM bank per j, 1 DVE copy each ----
            ET = [
                et_pool.tile([P, N], F16, name=f"et_{r}_{j}", tag="et")
                for j in range(FB)
            ]
            for j in range(FB):
                pt = pt_pool.tile([P, N], F16, name=f"pt_{r}_{j}", tag="pt")
                for c in range(CH):
                    for u in range(4):
                        nc.tensor.transpose(
                            pt[:, 512 * c + P * u : 512 * c + P * (u + 1)],
                            ens[c][:, F * u + P * j : F * u + P * (j + 1)],
                            ident,
                        )
                nc.vector.tensor_copy(ET[j], pt)

            # ---- matmul 1: eT[j] = (E @ W).T chunks  [128 g, 1024 n] ----
            E2 = []
            for j in range(FB):
                e2 = e2_pool.tile([P, N], F16, name=f"e2_{r}_{j}", tag="e2")
                for c in range(CH):
                    pe = pe_pool.tile([P, 512], F32, name=f"pe_{r}_{j}_{c}", tag="pe")
                    for k in range(FB):
                        nc.tensor.matmul(
                            pe,
                            lhsT=WT[r][k][:, P * j : P * (j + 1)],
                            rhs=ET[k][:, 512 * c : 512 * (c + 1)],
                            start=(k == 0),
                            stop=(k == FB - 1),
                        )
                    nc.vector.tensor_copy(e2[:, 512 * c : 512 * (c + 1)], pe)
                E2.append(e2)

            # ---- matmul 2 + sigmoid + store, one 128-row output block at a time ----
            for i in range(NB):
                ps = ps_pool.tile([P, N], F32, name=f"ps_{r}_{i}", tag="ps")
                for c in range(CH):
                    for j in range(FB):
                        nc.tensor.matmul(
                            ps[:, 512 * c : 512 * (c + 1)],
                            lhsT=E2[j][:, P * i : P * (i + 1)],
                            rhs=ET[j][:, 512 * c : 512 * (c + 1)],
                            start=(j == 0),
                            stop=(j == FB - 1),
                        )
                sg = sig_pool.tile([P, N], F16, name=f"sg_{r}_{i}", tag="sg")
                nc.scalar.activation(sg, ps, mybir.ActivationFunctionType.Sigmoid)
                nc.sync.dma_start(out=y[r, P * i : P * (i + 1), :], in_=sg)

    nc.compile()
    return nc


def kernel(inputs: np.ndarray, Bs: np.ndarray) -> np.ndarray:
    inputs = np.ascontiguousarray(inputs, dtype=np.float32)
    Bs = np.ascontiguousarray(Bs, dtype=np.float32)
    if "nc" not in _cache:
        _cache["nc"] = build_bass()
    nc = _cache["nc"]
    in_maps = [{"x": inputs[c], "w": Bs} for c in range(NCORES)]
    res = run_bass_kernel_spmd(nc, in_maps, list(range(NCORES)))
    out = np.stack(
        [np.asarray(res.results[c]["y"], dtype=np.float32) for c in range(NCORES)],
        axis=0,
    )
    return out
